# revision 6
# baseline (speedup 1.0000x reference)
"""Grouped MoE MLP (64 experts) on 8 Trainium2 NeuronCores.

Strategy: expert parallelism. Each core owns 8 experts (size-sorted "snake"
assignment so every core gets the same per-slot padded token capacity and the
padding is tight). Host pre-lays-out every tensor so each device DMA is one
large fully-contiguous transfer (>=0.5 MB, 4-32 KB runs per partition):

    w1t[s] : [128 hi, KO*F]   (hi, k, f)   one 4 MB DMA per slot
    w2[s]  : [128 fi, FO*H]   (fi, fo, h)  one 4 MB DMA per slot
    xT[s]  : [128 hi, KO*Cmax] (hi, k, t)  one ~0.6 MB DMA per slot
    outT[s]: [128 oi, OO*Cmax] (oi, oo, t) one ~0.6 MB bf16 DMA per slot

Both matmuls keep weights stationary and stream tokens as the moving operand:

    hT[f, t]   = w1t[e] (stationary, [h,f] tiles) @ xT (moving, [h, t])
    hT         = gelu(hT)                     (ScalarE, PSUM f32 -> SBUF bf16)
    outT[o, t] = w2[e] (stationary, [f,o] tiles) @ hT (moving, [f, t])

Weights stream HBM->SBUF once per core (double-buffered), PSUM accumulates in
f32, output is written back bf16 and upcast + un-permuted on host.
"""

import numpy as np

NCORES = 8
SLOTS = 8  # experts per core
NE = 64
H = 1024
F = 2048
T = 16384
P = 128
KO = H // P  # 8  k-tiles for mm1 (contraction over H)
FO = F // P  # 16 f-tiles (mm1 output tiles / mm2 contraction)
OO = H // P  # 8  output h-tiles for mm2
NMAX = 512  # max moving-operand length (one fp32 PSUM bank)

ACT_FN = "Gelu"  # overridable for CoreSim tests (Gelu not implemented there)

_prog_cache = {}


def _build_program(C):
    """Build the SPMD Bass program for per-slot token capacities C (len SLOTS)."""
    from contextlib import ExitStack

    import concourse.tile as tile
    from concourse import bacc, mybir
    from concourse.bass import MemorySpace

    bf16 = mybir.dt.bfloat16
    f32 = mybir.dt.float32
    Cmax = int(max(C))
    CmaxB = min(Cmax, NMAX)  # chunked tile width

    nc = bacc.Bacc("TRN2", target_bir_lowering=False, debug=False, num_devices=NCORES)
    w1t_d = nc.dram_tensor("w1t", [SLOTS, P, KO * F], bf16, kind="ExternalInput").ap()
    w2_d = nc.dram_tensor("w2", [SLOTS, P, FO * H], bf16, kind="ExternalInput").ap()
    xT_d = nc.dram_tensor("xT", [SLOTS, P, KO * Cmax], bf16, kind="ExternalInput").ap()
    outT_d = nc.dram_tensor(
        "outT", [SLOTS, P, OO * Cmax], bf16, kind="ExternalOutput"
    ).ap()

    with tile.TileContext(nc) as tc, ExitStack() as ctx:
        w1_pool = ctx.enter_context(tc.tile_pool(name="w1", bufs=2))
        w2_pool = ctx.enter_context(tc.tile_pool(name="w2", bufs=2))
        x_pool = ctx.enter_context(tc.tile_pool(name="x", bufs=2))
        h_pool = ctx.enter_context(tc.tile_pool(name="h", bufs=2))
        o_pool = ctx.enter_context(tc.tile_pool(name="o", bufs=2))
        ph_pool = ctx.enter_context(
            tc.tile_pool(name="ph", bufs=3, space=MemorySpace.PSUM)
        )
        po_pool = ctx.enter_context(
            tc.tile_pool(name="po", bufs=3, space=MemorySpace.PSUM)
        )

        for j in range(SLOTS):
            Cj = int(C[j])
            x_sb = x_pool.tile([P, KO * Cmax], bf16, tag="x")
            nc.sync.dma_start(x_sb, xT_d[j])
            w1_sb = w1_pool.tile([P, KO * F], bf16, tag="w1")
            if j == 0:
                # chunk per k so mm1 can start after the first 0.5 MB lands
                for k in range(KO):
                    nc.sync.dma_start(
                        w1_sb[:, k * F : (k + 1) * F], w1t_d[j, :, k * F : (k + 1) * F]
                    )
            else:
                nc.sync.dma_start(w1_sb, w1t_d[j])
            w2_sb = w2_pool.tile([P, FO * H], bf16, tag="w2")
            nc.sync.dma_start(w2_sb, w2_d[j])
            o_sb = o_pool.tile([P, OO * Cmax], bf16, tag="o")

            for nb in range(0, Cj, NMAX):
                NB = min(NMAX, Cj - nb)
                h_sb = h_pool.tile([P, FO * CmaxB], bf16, tag="h")
                for fo in range(FO):
                    ph = ph_pool.tile([P, NMAX], f32, tag="ph")
                    for k in range(KO):
                        nc.tensor.matmul(
                            ph[:, :NB],
                            w1_sb[:, k * F + fo * P : k * F + (fo + 1) * P],
                            x_sb[:, k * Cmax + nb : k * Cmax + nb + NB],
                            start=(k == 0),
                            stop=(k == KO - 1),
                        )
                    nc.scalar.activation(
                        h_sb[:, fo * CmaxB : fo * CmaxB + NB],
                        ph[:, :NB],
                        getattr(mybir.ActivationFunctionType, ACT_FN),
                    )
                for oo in range(OO):
                    po = po_pool.tile([P, NMAX], f32, tag="po")
                    for fo in range(FO):
                        nc.tensor.matmul(
                            po[:, :NB],
                            w2_sb[:, fo * H + oo * P : fo * H + (oo + 1) * P],
                            h_sb[:, fo * CmaxB : fo * CmaxB + NB],
                            start=(fo == 0),
                            stop=(fo == FO - 1),
                        )
                    nc.vector.tensor_copy(
                        o_sb[:, oo * Cmax + nb : oo * Cmax + nb + NB], po[:, :NB]
                    )
                    if j == SLOTS - 1 and nb + NB >= Cj:
                        # stream the final slot's output per-oo to shorten the tail
                        nc.sync.dma_start(
                            outT_d[j, :, oo * Cmax : (oo + 1) * Cmax],
                            o_sb[:, oo * Cmax : (oo + 1) * Cmax],
                        )
            if j != SLOTS - 1:
                nc.sync.dma_start(outT_d[j], o_sb)

    nc.compile()
    return nc


def _get_program(C):
    key = tuple(int(c) for c in C)
    if key not in _prog_cache:
        _prog_cache[key] = _build_program(key)
    return _prog_cache[key]


def plan(sizes):
    """Expert->core/slot assignment + slot capacities from token counts."""
    sizes = np.asarray(sizes, np.int64)
    assert sizes.shape == (NE,) and sizes.sum() == T
    order = np.argsort(-sizes, kind="stable")  # descending
    # expert_of[core][slot]
    expert_of = [[int(order[s * NCORES + c]) for s in range(SLOTS)] for c in range(NCORES)]
    C = []
    for s in range(SLOTS):
        m = max(int(sizes[order[s * NCORES + c]]) for c in range(NCORES))
        C.append(max(16, -(-m // 8) * 8))  # round up to multiple of 8, min 16
    return expert_of, C


def prepare_inputs(x, w1, w2, sizes, expert_of, C):
    """Host-side shard/pad/transpose/cast. Returns per-core input maps."""
    import ml_dtypes

    bf16 = ml_dtypes.bfloat16
    x = np.asarray(x, np.float32)
    tok_offs = np.concatenate([[0], np.cumsum(sizes)]).astype(np.int64)
    w1_bf = np.asarray(w1, np.float32).astype(bf16)  # [NE, F, H]
    w2_bf = np.asarray(w2, np.float32).astype(bf16)  # [NE, F, H]
    Cmax = int(max(C))

    in_maps = []
    for c in range(NCORES):
        experts = expert_of[c]
        # w1t: [S, hi, k, f] flattened to [S, 128, KO*F]
        w1t_c = np.ascontiguousarray(
            w1_bf[experts].transpose(0, 2, 1).reshape(SLOTS, KO, P, F).transpose(0, 2, 1, 3)
        ).reshape(SLOTS, P, KO * F)
        # w2: [S, fi, fo, h] flattened to [S, 128, FO*H]
        w2_c = np.ascontiguousarray(
            w2_bf[experts].reshape(SLOTS, FO, P, H).transpose(0, 2, 1, 3)
        ).reshape(SLOTS, P, FO * H)
        # xT: [S, hi, k, t] flattened to [S, 128, KO*Cmax]
        xT_c = np.zeros((SLOTS, P, KO, Cmax), np.float32)
        for s, e in enumerate(experts):
            n = int(sizes[e])
            xe = x[tok_offs[e] : tok_offs[e] + n]  # [n, H]
            xT_c[s, :, :, :n] = xe.T.reshape(KO, P, n).transpose(1, 0, 2)
        xT_c = xT_c.reshape(SLOTS, P, KO * Cmax).astype(bf16)
        in_maps.append({"w1t": w1t_c, "w2": w2_c, "xT": xT_c})
    return in_maps


def scatter_output(results, sizes, expert_of, C):
    """Gather per-core transposed outputs back into the full [T, H] f32 output."""
    tok_offs = np.concatenate([[0], np.cumsum(sizes)]).astype(np.int64)
    Cmax = int(max(C))
    out = np.empty((T, H), np.float32)
    for c in range(NCORES):
        # [S, oi, oo, t] -> per expert [H, n] -> [n, H]
        outT_c = np.asarray(results[c]["outT"]).reshape(SLOTS, P, OO, Cmax)
        for s, e in enumerate(expert_of[c]):
            n = int(sizes[e])
            blk = outT_c[s, :, :, :n].astype(np.float32)  # [oi, oo, n]
            out[tok_offs[e] : tok_offs[e] + n] = (
                blk.transpose(1, 0, 2).reshape(H, n).T
            )
    return out


LAST_RUN = None  # BassKernelResults from the most recent kernel() call


def kernel(x, w1, w2, tokens_per_expert):
    global LAST_RUN
    from concourse import bass_utils

    sizes = np.asarray(tokens_per_expert, np.int64)
    expert_of, C = plan(sizes)
    nc = _get_program(C)
    in_maps = prepare_inputs(x, w1, w2, sizes, expert_of, C)
    res = bass_utils.run_bass_kernel_spmd(nc, in_maps, core_ids=list(range(NCORES)))
    LAST_RUN = res
    return scatter_output(res.results, sizes, expert_of, C)


# revision 8
# speedup vs baseline: 1.0113x; 1.0113x over previous
"""Grouped MoE MLP (64 experts) on 8 Trainium2 NeuronCores.

Strategy: expert parallelism. Each core owns 8 experts (size-sorted "snake"
assignment so every core gets the same per-slot padded token capacity and the
padding is tight). Host pre-lays-out every tensor so each device DMA is one
large fully-contiguous transfer:

    w1t[s] : [128 hi, FO*KO*128] (hi, fo, k, f') one 4 MB DMA per slot
    w2[s]  : [128 fi, FO*H]      (fi, fo, h)     one 4 MB DMA per slot
    xT[s]  : [128 hi, KO*Cmax]   (hi, k, t)      one ~0.6 MB DMA per slot
    outT[s]: [128 oi, OO*Cmax]   (oi, oo, t)     one ~0.6 MB bf16 DMA per slot

w1 is laid out fo-major so slot 0 can stream it in 16 fo-chunks and mm1's
fo-groups consume chunks in arrival order (hides the startup weight DMA).
Slot 0's mm2 likewise runs fo-outer over 5 concurrent PSUM groups in two
passes so it consumes w2's fo-chunks as they arrive.

Both matmuls keep weights stationary and stream tokens as the moving operand:

    hT[f, t]   = w1t[e] (stationary, [h,f] tiles) @ xT (moving, [h, t])
    hT         = gelu(hT)                     (ScalarE, PSUM f32 -> SBUF bf16)
    outT[o, t] = w2[e] (stationary, [f,o] tiles) @ hT (moving, [f, t])

Weights stream HBM->SBUF once per core (double-buffered), PSUM accumulates in
f32, output is written back bf16 and upcast + un-permuted on host.
"""

import numpy as np

NCORES = 8
SLOTS = 8  # experts per core
NE = 64
H = 1024
F = 2048
T = 16384
P = 128
KO = H // P  # 8  k-tiles for mm1 (contraction over H)
FO = F // P  # 16 f-tiles (mm1 output tiles / mm2 contraction)
OO = H // P  # 8  output h-tiles for mm2
NMAX = 512  # max moving-operand length (one fp32 PSUM bank)
POG = 5  # concurrent mm2 PSUM groups for slot 0's fo-outer passes

ACT_FN = "Gelu"  # overridable for CoreSim tests (Gelu not implemented there)

_prog_cache = {}


def _build_program(C):
    """Build the SPMD Bass program for per-slot token capacities C (len SLOTS)."""
    from contextlib import ExitStack

    import concourse.tile as tile
    from concourse import bacc, mybir
    from concourse.bass import MemorySpace

    bf16 = mybir.dt.bfloat16
    f32 = mybir.dt.float32
    Cmax = int(max(C))
    CmaxB = min(Cmax, NMAX)  # chunked tile width

    nc = bacc.Bacc("TRN2", target_bir_lowering=False, debug=False, num_devices=NCORES)
    w1t_d = nc.dram_tensor("w1t", [SLOTS, P, FO * KO * P], bf16, kind="ExternalInput").ap()
    w2_d = nc.dram_tensor("w2", [SLOTS, P, FO * H], bf16, kind="ExternalInput").ap()
    xT_d = nc.dram_tensor("xT", [SLOTS, P, KO * Cmax], bf16, kind="ExternalInput").ap()
    outT_d = nc.dram_tensor(
        "outT", [SLOTS, P, OO * Cmax], bf16, kind="ExternalOutput"
    ).ap()

    def w1s(w1_sb, fo, k):  # stationary [128 hi, 128 f'] tile for (fo, k)
        base = fo * KO * P + k * P
        return w1_sb[:, base : base + P]

    def w2s(w2_sb, fo, oo):  # stationary [128 fi, 128 h'] tile for (fo, oo)
        base = fo * H + oo * P
        return w2_sb[:, base : base + P]

    with tile.TileContext(nc) as tc, ExitStack() as ctx:
        w1_pool = ctx.enter_context(tc.tile_pool(name="w1", bufs=2))
        w2_pool = ctx.enter_context(tc.tile_pool(name="w2", bufs=2))
        x_pool = ctx.enter_context(tc.tile_pool(name="x", bufs=2))
        h_pool = ctx.enter_context(tc.tile_pool(name="h", bufs=2))
        o_pool = ctx.enter_context(tc.tile_pool(name="o", bufs=2))
        ph_pool = ctx.enter_context(
            tc.tile_pool(name="ph", bufs=3, space=MemorySpace.PSUM)
        )
        po_pool = ctx.enter_context(
            tc.tile_pool(name="po", bufs=POG, space=MemorySpace.PSUM)
        )

        for j in range(SLOTS):
            Cj = int(C[j])
            x_sb = x_pool.tile([P, KO * Cmax], bf16, tag="x")
            nc.sync.dma_start(x_sb, xT_d[j])
            w1_sb = w1_pool.tile([P, FO * KO * P], bf16, tag="w1")
            w2_sb = w2_pool.tile([P, FO * H], bf16, tag="w2")
            if j == 0:
                # fo-chunked so mm1 fo-groups consume chunks in arrival order
                for fo in range(FO):
                    nc.sync.dma_start(
                        w1_sb[:, fo * KO * P : (fo + 1) * KO * P],
                        w1t_d[j, :, fo * KO * P : (fo + 1) * KO * P],
                    )
                for fo in range(FO):
                    nc.sync.dma_start(
                        w2_sb[:, fo * H : (fo + 1) * H],
                        w2_d[j, :, fo * H : (fo + 1) * H],
                    )
            else:
                nc.sync.dma_start(w1_sb, w1t_d[j])
                nc.sync.dma_start(w2_sb, w2_d[j])
            o_sb = o_pool.tile([P, OO * Cmax], bf16, tag="o")

            for nb in range(0, Cj, NMAX):
                NB = min(NMAX, Cj - nb)
                h_sb = h_pool.tile([P, FO * CmaxB], bf16, tag="h")
                for fo in range(FO):
                    ph = ph_pool.tile([P, NMAX], f32, tag="ph")
                    for k in range(KO):
                        nc.tensor.matmul(
                            ph[:, :NB],
                            w1s(w1_sb, fo, k),
                            x_sb[:, k * Cmax + nb : k * Cmax + nb + NB],
                            start=(k == 0),
                            stop=(k == KO - 1),
                        )
                    nc.scalar.activation(
                        h_sb[:, fo * CmaxB : fo * CmaxB + NB],
                        ph[:, :NB],
                        getattr(mybir.ActivationFunctionType, ACT_FN),
                    )

                def emit_out(oo, po):
                    nc.vector.tensor_copy(
                        o_sb[:, oo * Cmax + nb : oo * Cmax + nb + NB], po[:, :NB]
                    )

                if j == 0:
                    # fo-outer over POG concurrent PSUM groups: consumes w2
                    # fo-chunks as they arrive during startup
                    for ob in range(0, OO, POG):
                        oos = range(ob, min(ob + POG, OO))
                        pos = {
                            oo: po_pool.tile([P, NMAX], f32, tag="po", name=f"po{oo}")
                            for oo in oos
                        }
                        for fo in range(FO):
                            for oo in oos:
                                nc.tensor.matmul(
                                    pos[oo][:, :NB],
                                    w2s(w2_sb, fo, oo),
                                    h_sb[:, fo * CmaxB : fo * CmaxB + NB],
                                    start=(fo == 0),
                                    stop=(fo == FO - 1),
                                )
                        for oo in oos:
                            emit_out(oo, pos[oo])
                else:
                    for oo in range(OO):
                        po = po_pool.tile([P, NMAX], f32, tag="po")
                        for fo in range(FO):
                            nc.tensor.matmul(
                                po[:, :NB],
                                w2s(w2_sb, fo, oo),
                                h_sb[:, fo * CmaxB : fo * CmaxB + NB],
                                start=(fo == 0),
                                stop=(fo == FO - 1),
                            )
                        emit_out(oo, po)
            nc.sync.dma_start(outT_d[j], o_sb)

    nc.compile()
    return nc


def _get_program(C):
    key = tuple(int(c) for c in C)
    if key not in _prog_cache:
        _prog_cache[key] = _build_program(key)
    return _prog_cache[key]


def plan(sizes):
    """Expert->core/slot assignment + slot capacities from token counts."""
    sizes = np.asarray(sizes, np.int64)
    assert sizes.shape == (NE,) and sizes.sum() == T
    order = np.argsort(-sizes, kind="stable")  # descending
    # expert_of[core][slot]
    expert_of = [[int(order[s * NCORES + c]) for s in range(SLOTS)] for c in range(NCORES)]
    C = []
    for s in range(SLOTS):
        m = max(int(sizes[order[s * NCORES + c]]) for c in range(NCORES))
        C.append(max(16, -(-m // 8) * 8))  # round up to multiple of 8, min 16
    return expert_of, C


def prepare_inputs(x, w1, w2, sizes, expert_of, C):
    """Host-side shard/pad/transpose/cast. Returns per-core input maps."""
    import ml_dtypes

    bf16 = ml_dtypes.bfloat16
    x = np.asarray(x, np.float32)
    tok_offs = np.concatenate([[0], np.cumsum(sizes)]).astype(np.int64)
    w1_bf = np.asarray(w1, np.float32).astype(bf16)  # [NE, F, H]
    w2_bf = np.asarray(w2, np.float32).astype(bf16)  # [NE, F, H]
    Cmax = int(max(C))

    in_maps = []
    for c in range(NCORES):
        experts = expert_of[c]
        # w1t: [S, hi, fo, k, f'] flattened to [S, 128, FO*KO*128]
        # w1[e] is [F, H]; stationary tile (fo,k) is w1[e][fo*128+f', k*128+hi].T
        w1t_c = np.ascontiguousarray(
            w1_bf[experts]
            .reshape(SLOTS, FO, P, KO, P)  # [s, fo, f', k, hi]
            .transpose(0, 4, 1, 3, 2)  # [s, hi, fo, k, f']
        ).reshape(SLOTS, P, FO * KO * P)
        # w2: [S, fi, fo, h] flattened to [S, 128, FO*H]
        w2_c = np.ascontiguousarray(
            w2_bf[experts].reshape(SLOTS, FO, P, H).transpose(0, 2, 1, 3)
        ).reshape(SLOTS, P, FO * H)
        # xT: [S, hi, k, t] flattened to [S, 128, KO*Cmax]
        xT_c = np.zeros((SLOTS, P, KO, Cmax), np.float32)
        for s, e in enumerate(experts):
            n = int(sizes[e])
            xe = x[tok_offs[e] : tok_offs[e] + n]  # [n, H]
            xT_c[s, :, :, :n] = xe.T.reshape(KO, P, n).transpose(1, 0, 2)
        xT_c = xT_c.reshape(SLOTS, P, KO * Cmax).astype(bf16)
        in_maps.append({"w1t": w1t_c, "w2": w2_c, "xT": xT_c})
    return in_maps


def scatter_output(results, sizes, expert_of, C):
    """Gather per-core transposed outputs back into the full [T, H] f32 output."""
    tok_offs = np.concatenate([[0], np.cumsum(sizes)]).astype(np.int64)
    Cmax = int(max(C))
    out = np.empty((T, H), np.float32)
    for c in range(NCORES):
        # [S, oi, oo, t] -> per expert [H, n] -> [n, H]
        outT_c = np.asarray(results[c]["outT"]).reshape(SLOTS, P, OO, Cmax)
        for s, e in enumerate(expert_of[c]):
            n = int(sizes[e])
            blk = outT_c[s, :, :, :n].astype(np.float32)  # [oi, oo, n]
            out[tok_offs[e] : tok_offs[e] + n] = (
                blk.transpose(1, 0, 2).reshape(H, n).T
            )
    return out


LAST_RUN = None  # BassKernelResults from the most recent kernel() call


def kernel(x, w1, w2, tokens_per_expert):
    global LAST_RUN
    from concourse import bass_utils

    sizes = np.asarray(tokens_per_expert, np.int64)
    expert_of, C = plan(sizes)
    nc = _get_program(C)
    in_maps = prepare_inputs(x, w1, w2, sizes, expert_of, C)
    res = bass_utils.run_bass_kernel_spmd(nc, in_maps, core_ids=list(range(NCORES)))
    LAST_RUN = res
    return scatter_output(res.results, sizes, expert_of, C)
